# revision 15
# baseline (speedup 1.0000x reference)
"""AdMSoftmax loss on 8 TRN2 NeuronCores.

Math (reference, all f32):
    num_i  = S*(wf[i, y_i] - M)
    L_i    = num_i - log(exp(num_i) + sum_j exp(S*wf[i,j]) - exp(S*wf[i,y_i]))
    out    = -mean_i(L_i)

With S=30 and wf ~ N(0,1), exp(S*wf) overflows f32 (max S*wf ~ 157), so
the reference's f32 pipeline produces inf row sums and (for rows whose
label column itself overflows) inf - inf = nan; the reference output is
then nan.  To reproduce those IEEE-f32 semantics bit-faithfully we:

  1. On device (the O(B*C) work, data-parallel over 8 cores, 512 rows
     each): compute the SHIFTED row sums R_i = sum_j exp(S*wf_ij - SHIFT)
     with SHIFT=90.  Max shifted exponent ~ 157-90 = 67 -> e^67 ~ 1.2e29,
     so R_i never overflows f32, and the true log-sum-exp is
     SHIFT + log(R_i).  (Terms with S*wf < ~3 underflow to 0 in the
     shifted sum; their relative contribution vs the row max term
     exp(S*max_row - SHIFT) is < 20000*e^-87/e^(S*max_row-90), i.e.
     negligible whenever the row max exceeds ~0.2 -- guaranteed at these
     shapes where row maxima of 20000 N(0,1) draws concentrate near 3.9.)
     One ScalarEngine ACTIVATE(Exp, scale=S, bias=-SHIFT, accum_out=...)
     per tile fuses the exp and the free-dim row sum.
  2. On host (O(B) work): reconstruct the f32 row sum the reference saw
     as float32(e^SHIFT * R_i) evaluated in f64 -> inf exactly where the
     reference's f32 sum overflowed -- then replay the reference's per-row
     f32 epilogue (gather, exp, sub, log, mean) in numpy f32, which
     follows the same IEEE rules (inf/nan propagation included).

The final 8-way combine is a host-side mean over the gathered per-row
values (the all-reduce of the sharding hint, done on 4096 floats).
"""

import numpy as np

S = 30.0
M = 0.4
B = 4096
C = 20000
NCORES = 8
ROWS = B // NCORES       # rows per core = 512
P = 128                  # SBUF partitions
TILES = ROWS // P        # row tiles per core = 4
import os as _os
CHUNK = int(_os.environ.get("K_CHUNK", "4000"))  # columns per ACTIVATE/DMA chunk
BUFS = int(_os.environ.get("K_BUFS", "5"))       # wf tile double-buffering depth
NCH = C // CHUNK         # chunks per row tile
SHIFT = 90.0             # exp argument shift keeping f32 finite

_CACHE = {}
LAST_RESULTS = None      # BassKernelResults of the most recent run (for test.py)


def _build_graph():
    import concourse.tile as tile
    from concourse import bacc, mybir

    nc = bacc.Bacc(
        "TRN2",
        target_bir_lowering=False,
        debug=False,
        enable_asserts=False,
        num_devices=NCORES,
    )
    wf = nc.dram_tensor("wf", [ROWS, C], mybir.dt.float32, kind="ExternalInput").ap()
    out = nc.dram_tensor(
        "out", [P, TILES], mybir.dt.float32, kind="ExternalOutput"
    ).ap()

    with tile.TileContext(nc) as tc:
        with (
            tc.tile_pool(name="wfp", bufs=BUFS) as wfp,
            tc.tile_pool(name="accp", bufs=1) as accp,
            tc.tile_pool(name="outp", bufs=1) as outp,
        ):
            acc = accp.tile([P, TILES * NCH + 1], mybir.dt.float32)
            osb = outp.tile([P, TILES], mybir.dt.float32)
            bias_t = accp.tile([P, 1], mybir.dt.float32)
            nc.any.memset(bias_t[:, :], -SHIFT)
            # Column chunking: uniform CHUNK-wide pieces, except the very
            # last chunk of the last row tile is split in half so the
            # final ACTIVATE (which sits in the kernel tail after the
            # last DMA) is half as long.
            spans = []  # (tile, col_start, col_len, acc_col)
            acc_cols = 0
            for t in range(TILES):
                for ci in range(NCH):
                    c0, cl = ci * CHUNK, CHUNK
                    last = t == TILES - 1 and ci == NCH - 1
                    if last and CHUNK % 2 == 0 and _os.environ.get("K_SPLIT_TAIL", "0") == "1":
                        spans.append((t, c0, cl // 2, acc_cols))
                        acc_cols += 1
                        c0, cl = c0 + cl // 2, cl // 2
                    spans.append((t, c0, cl, acc_cols))
                    acc_cols += 1

            dma_engines = [nc.sync, nc.scalar]
            ring_mode = int(_os.environ.get("K_RING_MODE", "0"))
            for k, (t, c0, cl, ak) in enumerate(spans):
                if ring_mode == 1:
                    eng = t  # per row-tile
                elif ring_mode == 2:
                    eng = 0 if k < len(spans) // 2 else 1
                else:
                    eng = k  # alternate per chunk
                wt = wfp.tile([P, CHUNK], mybir.dt.float32)
                dma_engines[eng % len(dma_engines)].dma_start(
                    out=wt[:, :cl],
                    in_=wf[t * P : (t + 1) * P, c0 : c0 + cl],
                )
                nc.scalar.activation(
                    out=wt[:, :cl],
                    in_=wt[:, :cl],
                    func=mybir.ActivationFunctionType.Exp,
                    scale=S,
                    bias=bias_t[:, :],
                    accum_out=acc[:, ak : ak + 1],
                )
            for t in range(TILES):
                lo = min(ak for (tt, _, _, ak) in spans if tt == t)
                hi = max(ak for (tt, _, _, ak) in spans if tt == t)
                nc.vector.tensor_reduce(
                    osb[:, t : t + 1],
                    acc[:, lo : hi + 1],
                    axis=mybir.AxisListType.X,
                    op=mybir.AluOpType.add,
                )
            nc.sync.dma_start(out=out[:, :], in_=osb[:, :])
    nc.compile()
    return nc


def _get_graph():
    if "nc" not in _CACHE:
        _CACHE["nc"] = _build_graph()
    return _CACHE["nc"]


def kernel(wf: np.ndarray, labels: np.ndarray) -> np.ndarray:
    global LAST_RESULTS
    from concourse import bass_utils

    wf = np.ascontiguousarray(np.asarray(wf, dtype=np.float32))
    lab = np.asarray(labels).astype(np.int64)
    assert wf.shape == (B, C) and lab.shape == (B,)

    nc = _get_graph()
    in_maps = [{"wf": wf[i * ROWS : (i + 1) * ROWS]} for i in range(NCORES)]
    res = bass_utils.run_bass_kernel_spmd(nc, in_maps, core_ids=list(range(NCORES)))
    LAST_RESULTS = res

    # res.results[i]["out"][p, t] = shifted row sum of shard row t*P + p.
    R = np.concatenate(
        [res.results[i]["out"].T.reshape(ROWS) for i in range(NCORES)]
    ).astype(np.float64)  # [B]

    # Host epilogue: replay the reference's f32 arithmetic exactly.
    rows = np.arange(B)
    tgt = wf[rows, lab]                                     # [B] f32
    with np.errstate(over="ignore", invalid="ignore", divide="ignore"):
        rowsum32 = (np.exp(np.float64(SHIFT)) * R).astype(np.float32)  # inf on overflow
        exp_lab = np.exp(np.float32(S) * tgt)               # f32, inf where S*t>~88.7
        num = np.float32(S) * (tgt - np.float32(M))         # f32
        excl = rowsum32 - exp_lab
        denom = np.exp(num) + excl
        L = num - np.log(denom)
        out = -np.mean(L, dtype=np.float32)
    return np.float32(out)


# revision 16
# speedup vs baseline: 1.0300x; 1.0300x over previous
"""AdMSoftmax loss on 8 TRN2 NeuronCores.

Math (reference, all f32):
    num_i  = S*(wf[i, y_i] - M)
    L_i    = num_i - log(exp(num_i) + sum_j exp(S*wf[i,j]) - exp(S*wf[i,y_i]))
    out    = -mean_i(L_i)

With S=30 and wf ~ N(0,1), exp(S*wf) overflows f32 (max S*wf ~ 157), so
the reference's f32 pipeline produces inf row sums and (for rows whose
label column itself overflows) inf - inf = nan; the reference output is
then nan.  To reproduce those IEEE-f32 semantics bit-faithfully we:

  1. On device (the O(B*C) work, data-parallel over 8 cores, 512 rows
     each): compute the SHIFTED row sums R_i = sum_j exp(S*wf_ij - SHIFT)
     with SHIFT=90.  Max shifted exponent ~ 157-90 = 67 -> e^67 ~ 1.2e29,
     so R_i never overflows f32, and the true log-sum-exp is
     SHIFT + log(R_i).  (Terms with S*wf < ~3 underflow to 0 in the
     shifted sum; their relative contribution vs the row max term
     exp(S*max_row - SHIFT) is < 20000*e^-87/e^(S*max_row-90), i.e.
     negligible whenever the row max exceeds ~0.2 -- guaranteed at these
     shapes where row maxima of 20000 N(0,1) draws concentrate near 3.9.)
     One ScalarEngine ACTIVATE(Exp, scale=S, bias=-SHIFT, accum_out=...)
     per tile fuses the exp and the free-dim row sum.
  2. On host (O(B) work): reconstruct the f32 row sum the reference saw
     as float32(e^SHIFT * R_i) evaluated in f64 -> inf exactly where the
     reference's f32 sum overflowed -- then replay the reference's per-row
     f32 epilogue (gather, exp, sub, log, mean) in numpy f32, which
     follows the same IEEE rules (inf/nan propagation included).

The final 8-way combine is a host-side mean over the gathered per-row
values (the all-reduce of the sharding hint, done on 4096 floats).
"""

import numpy as np

S = 30.0
M = 0.4
B = 4096
C = 20000
NCORES = 8
ROWS = B // NCORES       # rows per core = 512
P = 128                  # SBUF partitions
TILES = ROWS // P        # row tiles per core = 4
import os as _os
CHUNK = int(_os.environ.get("K_CHUNK", "4000"))  # columns per ACTIVATE/DMA chunk
BUFS = int(_os.environ.get("K_BUFS", "5"))       # wf tile double-buffering depth
NCH = C // CHUNK         # chunks per row tile
SHIFT = 90.0             # exp argument shift keeping f32 finite

_CACHE = {}
LAST_RESULTS = None      # BassKernelResults of the most recent run (for test.py)


def _build_graph():
    import concourse.tile as tile
    from concourse import bacc, mybir

    nc = bacc.Bacc(
        "TRN2",
        target_bir_lowering=False,
        debug=False,
        enable_asserts=False,
        num_devices=NCORES,
    )
    wf = nc.dram_tensor("wf", [ROWS, C], mybir.dt.float32, kind="ExternalInput").ap()
    out = nc.dram_tensor(
        "out", [P, TILES], mybir.dt.float32, kind="ExternalOutput"
    ).ap()

    with tile.TileContext(nc) as tc:
        with (
            tc.tile_pool(name="wfp", bufs=BUFS) as wfp,
            tc.tile_pool(name="accp", bufs=1) as accp,
            tc.tile_pool(name="outp", bufs=1) as outp,
        ):
            acc = accp.tile([P, TILES * NCH + 1], mybir.dt.float32)
            osb = outp.tile([P, TILES], mybir.dt.float32)
            bias_t = accp.tile([P, 1], mybir.dt.float32)
            nc.any.memset(bias_t[:, :], -SHIFT)
            # Column chunking: uniform CHUNK-wide pieces. (Optionally the
            # last chunk can be split in half to shorten the final
            # ACTIVATE in the kernel tail; measured as a wash, off by
            # default.)  Chunk DMAs alternate between the two HWDGE
            # rings (nc.sync / nc.scalar) so descriptor issue and
            # completion latencies of consecutive transfers overlap --
            # worth ~12us over a single ring at these sizes.
            spans = []  # (tile, col_start, col_len, acc_col)
            acc_cols = 0
            for t in range(TILES):
                for ci in range(NCH):
                    c0, cl = ci * CHUNK, CHUNK
                    last = t == TILES - 1 and ci == NCH - 1
                    if last and CHUNK % 2 == 0 and _os.environ.get("K_SPLIT_TAIL", "0") == "1":
                        spans.append((t, c0, cl // 2, acc_cols))
                        acc_cols += 1
                        c0, cl = c0 + cl // 2, cl // 2
                    spans.append((t, c0, cl, acc_cols))
                    acc_cols += 1

            dma_engines = [nc.sync, nc.scalar]
            ring_mode = int(_os.environ.get("K_RING_MODE", "0"))
            for k, (t, c0, cl, ak) in enumerate(spans):
                if ring_mode == 1:
                    eng = t  # per row-tile
                elif ring_mode == 2:
                    eng = 0 if k < len(spans) // 2 else 1
                else:
                    eng = k  # alternate per chunk
                wt = wfp.tile([P, CHUNK], mybir.dt.float32)
                dma_engines[eng % len(dma_engines)].dma_start(
                    out=wt[:, :cl],
                    in_=wf[t * P : (t + 1) * P, c0 : c0 + cl],
                )
                nc.scalar.activation(
                    out=wt[:, :cl],
                    in_=wt[:, :cl],
                    func=mybir.ActivationFunctionType.Exp,
                    scale=S,
                    bias=bias_t[:, :],
                    accum_out=acc[:, ak : ak + 1],
                )
            for t in range(TILES):
                lo = min(ak for (tt, _, _, ak) in spans if tt == t)
                hi = max(ak for (tt, _, _, ak) in spans if tt == t)
                nc.vector.tensor_reduce(
                    osb[:, t : t + 1],
                    acc[:, lo : hi + 1],
                    axis=mybir.AxisListType.X,
                    op=mybir.AluOpType.add,
                )
            nc.sync.dma_start(out=out[:, :], in_=osb[:, :])
    nc.compile()
    return nc


def _get_graph():
    if "nc" not in _CACHE:
        _CACHE["nc"] = _build_graph()
    return _CACHE["nc"]


def kernel(wf: np.ndarray, labels: np.ndarray) -> np.ndarray:
    global LAST_RESULTS
    from concourse import bass_utils

    wf = np.ascontiguousarray(np.asarray(wf, dtype=np.float32))
    lab = np.asarray(labels).astype(np.int64)
    assert wf.shape == (B, C) and lab.shape == (B,)

    nc = _get_graph()
    in_maps = [{"wf": wf[i * ROWS : (i + 1) * ROWS]} for i in range(NCORES)]
    res = bass_utils.run_bass_kernel_spmd(nc, in_maps, core_ids=list(range(NCORES)))
    LAST_RESULTS = res

    # res.results[i]["out"][p, t] = shifted row sum of shard row t*P + p.
    R = np.concatenate(
        [res.results[i]["out"].T.reshape(ROWS) for i in range(NCORES)]
    ).astype(np.float64)  # [B]

    # Host epilogue: replay the reference's f32 arithmetic exactly.
    rows = np.arange(B)
    tgt = wf[rows, lab]                                     # [B] f32
    with np.errstate(over="ignore", invalid="ignore", divide="ignore"):
        rowsum32 = (np.exp(np.float64(SHIFT)) * R).astype(np.float32)  # inf on overflow
        exp_lab = np.exp(np.float32(S) * tgt)               # f32, inf where S*t>~88.7
        num = np.float32(S) * (tgt - np.float32(M))         # f32
        excl = rowsum32 - exp_lab
        denom = np.exp(num) + excl
        L = num - np.log(denom)
        out = -np.mean(L, dtype=np.float32)
    return np.float32(out)


# revision 21
# speedup vs baseline: 1.0325x; 1.0024x over previous
"""AdMSoftmax loss on 8 TRN2 NeuronCores.

Math (reference, all f32):
    num_i  = S*(wf[i, y_i] - M)
    L_i    = num_i - log(exp(num_i) + sum_j exp(S*wf[i,j]) - exp(S*wf[i,y_i]))
    out    = -mean_i(L_i)

With S=30 and wf ~ N(0,1), exp(S*wf) overflows f32 (max S*wf ~ 157), so
the reference's f32 pipeline produces inf row sums and (for rows whose
label column itself overflows) inf - inf = nan; the reference output is
then nan.  To reproduce those IEEE-f32 semantics bit-faithfully we:

  1. On device (the O(B*C) work, data-parallel over 8 cores, 512 rows
     each): compute the SHIFTED row sums R_i = sum_j exp(S*wf_ij - SHIFT)
     with SHIFT=90.  Max shifted exponent ~ 157-90 = 67 -> e^67 ~ 1.2e29,
     so R_i never overflows f32, and the true log-sum-exp is
     SHIFT + log(R_i).  (Terms with S*wf < ~3 underflow to 0 in the
     shifted sum; their relative contribution vs the row max term
     exp(S*max_row - SHIFT) is < 20000*e^-87/e^(S*max_row-90), i.e.
     negligible whenever the row max exceeds ~0.2 -- guaranteed at these
     shapes where row maxima of 20000 N(0,1) draws concentrate near 3.9.)
     One ScalarEngine ACTIVATE(Exp, scale=S, bias=-SHIFT, accum_out=...)
     per tile fuses the exp and the free-dim row sum.
  2. On host (O(B) work): reconstruct the f32 row sum the reference saw
     as float32(e^SHIFT * R_i) evaluated in f64 -> inf exactly where the
     reference's f32 sum overflowed -- then replay the reference's per-row
     f32 epilogue (gather, exp, sub, log, mean) in numpy f32, which
     follows the same IEEE rules (inf/nan propagation included).

The final 8-way combine is a host-side mean over the gathered per-row
values (the all-reduce of the sharding hint, done on 4096 floats).
"""

import numpy as np

S = 30.0
M = 0.4
B = 4096
C = 20000
NCORES = 8
ROWS = B // NCORES       # rows per core = 512
P = 128                  # SBUF partitions
TILES = ROWS // P        # row tiles per core = 4
import os as _os
CHUNK = int(_os.environ.get("K_CHUNK", "4000"))  # columns per ACTIVATE/DMA chunk
BUFS = int(_os.environ.get("K_BUFS", "5"))       # wf tile double-buffering depth
NCH = C // CHUNK         # chunks per row tile
SHIFT = 90.0             # exp argument shift keeping f32 finite

_CACHE = {}
LAST_RESULTS = None      # BassKernelResults of the most recent run (for test.py)


def _build_graph():
    import concourse.tile as tile
    from concourse import bacc, mybir

    nc = bacc.Bacc(
        "TRN2",
        target_bir_lowering=False,
        debug=False,
        enable_asserts=False,
        num_devices=NCORES,
    )
    wf = nc.dram_tensor("wf", [ROWS, C], mybir.dt.float32, kind="ExternalInput").ap()
    out = nc.dram_tensor(
        "out", [P, TILES], mybir.dt.float32, kind="ExternalOutput"
    ).ap()

    with tile.TileContext(nc) as tc:
        with (
            tc.tile_pool(name="wfp", bufs=BUFS) as wfp,
            tc.tile_pool(name="accp", bufs=1) as accp,
            tc.tile_pool(name="outp", bufs=1) as outp,
        ):
            osb = outp.tile([P, TILES], mybir.dt.float32)
            bias_t = accp.tile([P, 1], mybir.dt.float32)
            nc.any.memset(bias_t[:, :], -SHIFT)
            # Column chunking: uniform CHUNK-wide pieces. (Optionally the
            # last chunk can be split in half to shorten the final
            # ACTIVATE in the kernel tail; measured as a wash, off by
            # default.)  Chunk DMAs alternate between the two HWDGE
            # rings (nc.sync / nc.scalar) so descriptor issue and
            # completion latencies of consecutive transfers overlap --
            # worth ~12us over a single ring at these sizes.
            # Halve the last K_TAIL_HALVE full chunks: their EXPs run
            # after the DMA stream has finished (ACT is the critical
            # path in the kernel tail), and smaller chunks drain that
            # backlog sooner.
            tail_halve = int(_os.environ.get("K_TAIL_HALVE", "0"))
            spans = []  # (tile, col_start, col_len, acc_col)
            acc_cols = 0
            n_full = TILES * NCH
            for t in range(TILES):
                for ci in range(NCH):
                    c0, cl = ci * CHUNK, CHUNK
                    k_full = t * NCH + ci
                    if k_full >= n_full - tail_halve and CHUNK % 2 == 0:
                        spans.append((t, c0, cl // 2, acc_cols))
                        acc_cols += 1
                        c0, cl = c0 + cl // 2, cl // 2
                    spans.append((t, c0, cl, acc_cols))
                    acc_cols += 1

            acc = accp.tile([P, acc_cols], mybir.dt.float32)
            dma_engines = [nc.sync, nc.scalar]
            ring_mode = int(_os.environ.get("K_RING_MODE", "4"))
            for k, (t, c0, cl, ak) in enumerate(spans):
                if ring_mode == 1:
                    eng = t  # per row-tile
                elif ring_mode == 2:
                    eng = 0 if k < len(spans) // 2 else 1
                elif ring_mode >= 3:
                    # Alternate rings mid-stream, but issue the last
                    # (ring_mode) chunks from the sync ring: the scalar
                    # sequencer is in-order and busy running the EXPs, so
                    # its late DMA issues otherwise stall the pipe tail.
                    tail_n = ring_mode
                    eng = k if k < len(spans) - tail_n else 0
                else:
                    eng = k  # alternate per chunk
                wt = wfp.tile([P, CHUNK], mybir.dt.float32)
                dma_engines[eng % len(dma_engines)].dma_start(
                    out=wt[:, :cl],
                    in_=wf[t * P : (t + 1) * P, c0 : c0 + cl],
                )
                nc.scalar.activation(
                    out=wt[:, :cl],
                    in_=wt[:, :cl],
                    func=mybir.ActivationFunctionType.Exp,
                    scale=S,
                    bias=bias_t[:, :],
                    accum_out=acc[:, ak : ak + 1],
                )
            for t in range(TILES):
                lo = min(ak for (tt, _, _, ak) in spans if tt == t)
                hi = max(ak for (tt, _, _, ak) in spans if tt == t)
                nc.vector.tensor_reduce(
                    osb[:, t : t + 1],
                    acc[:, lo : hi + 1],
                    axis=mybir.AxisListType.X,
                    op=mybir.AluOpType.add,
                )
            nc.sync.dma_start(out=out[:, :], in_=osb[:, :])
    nc.compile()
    return nc


def _get_graph():
    if "nc" not in _CACHE:
        _CACHE["nc"] = _build_graph()
    return _CACHE["nc"]


def kernel(wf: np.ndarray, labels: np.ndarray) -> np.ndarray:
    global LAST_RESULTS
    from concourse import bass_utils

    wf = np.ascontiguousarray(np.asarray(wf, dtype=np.float32))
    lab = np.asarray(labels).astype(np.int64)
    assert wf.shape == (B, C) and lab.shape == (B,)

    nc = _get_graph()
    in_maps = [{"wf": wf[i * ROWS : (i + 1) * ROWS]} for i in range(NCORES)]
    res = bass_utils.run_bass_kernel_spmd(nc, in_maps, core_ids=list(range(NCORES)))
    LAST_RESULTS = res

    # res.results[i]["out"][p, t] = shifted row sum of shard row t*P + p.
    R = np.concatenate(
        [res.results[i]["out"].T.reshape(ROWS) for i in range(NCORES)]
    ).astype(np.float64)  # [B]

    # Host epilogue: replay the reference's f32 arithmetic exactly.
    rows = np.arange(B)
    tgt = wf[rows, lab]                                     # [B] f32
    with np.errstate(over="ignore", invalid="ignore", divide="ignore"):
        rowsum32 = (np.exp(np.float64(SHIFT)) * R).astype(np.float32)  # inf on overflow
        exp_lab = np.exp(np.float32(S) * tgt)               # f32, inf where S*t>~88.7
        num = np.float32(S) * (tgt - np.float32(M))         # f32
        excl = rowsum32 - exp_lab
        denom = np.exp(num) + excl
        L = num - np.log(denom)
        out = -np.mean(L, dtype=np.float32)
    return np.float32(out)


# revision 24
# speedup vs baseline: 1.0557x; 1.0225x over previous
"""AdMSoftmax loss on 8 TRN2 NeuronCores.

Math (reference, all f32):
    num_i  = S*(wf[i, y_i] - M)
    L_i    = num_i - log(exp(num_i) + sum_j exp(S*wf[i,j]) - exp(S*wf[i,y_i]))
    out    = -mean_i(L_i)

With S=30 and wf ~ N(0,1), exp(S*wf) overflows f32 (max S*wf ~ 157), so
the reference's f32 pipeline produces inf row sums and (for rows whose
label column itself overflows) inf - inf = nan; the reference output is
then nan.  To reproduce those IEEE-f32 semantics bit-faithfully we:

  1. On device (the O(B*C) work, data-parallel over 8 cores, 512 rows
     each): compute the SHIFTED row sums R_i = sum_j exp(S*wf_ij - SHIFT)
     with SHIFT=90.  Max shifted exponent ~ 157-90 = 67 -> e^67 ~ 1.2e29,
     so R_i never overflows f32, and the true log-sum-exp is
     SHIFT + log(R_i).  (Terms with S*wf < ~3 underflow to 0 in the
     shifted sum; their relative contribution vs the row max term
     exp(S*max_row - SHIFT) is < 20000*e^-87/e^(S*max_row-90), i.e.
     negligible whenever the row max exceeds ~0.2 -- guaranteed at these
     shapes where row maxima of 20000 N(0,1) draws concentrate near 3.9.)
     One ScalarEngine ACTIVATE(Exp, scale=S, bias=-SHIFT, accum_out=...)
     per tile fuses the exp and the free-dim row sum.
  2. On host (O(B) work): reconstruct the f32 row sum the reference saw
     as float32(e^SHIFT * R_i) evaluated in f64 -> inf exactly where the
     reference's f32 sum overflowed -- then replay the reference's per-row
     f32 epilogue (gather, exp, sub, log, mean) in numpy f32, which
     follows the same IEEE rules (inf/nan propagation included).

The final 8-way combine is a host-side mean over the gathered per-row
values (the all-reduce of the sharding hint, done on 4096 floats).
"""

import numpy as np

S = 30.0
M = 0.4
B = 4096
C = 20000
NCORES = 8
ROWS = B // NCORES       # rows per core = 512
P = 128                  # SBUF partitions
TILES = ROWS // P        # row tiles per core = 4
import os as _os
CHUNK = int(_os.environ.get("K_CHUNK", "4000"))  # columns per ACTIVATE/DMA chunk
BUFS = int(_os.environ.get("K_BUFS", "5"))       # wf tile double-buffering depth
NCH = C // CHUNK         # chunks per row tile
SHIFT = 90.0             # exp argument shift keeping f32 finite

_CACHE = {}
LAST_RESULTS = None      # BassKernelResults of the most recent run (for test.py)


def _build_graph():
    import concourse.tile as tile
    from concourse import bacc, mybir

    nc = bacc.Bacc(
        "TRN2",
        target_bir_lowering=False,
        debug=False,
        enable_asserts=False,
        num_devices=NCORES,
    )
    wf = nc.dram_tensor("wf", [ROWS, C], mybir.dt.float32, kind="ExternalInput").ap()
    out = nc.dram_tensor(
        "out", [P, TILES], mybir.dt.float32, kind="ExternalOutput"
    ).ap()

    with tile.TileContext(nc) as tc:
        with (
            tc.tile_pool(name="wfp", bufs=BUFS) as wfp,
            tc.tile_pool(name="accp", bufs=1) as accp,
            tc.tile_pool(name="outp", bufs=1) as outp,
        ):
            osb = outp.tile([P, TILES], mybir.dt.float32)
            bias_t = accp.tile([P, 1], mybir.dt.float32)
            nc.any.memset(bias_t[:, :], -SHIFT)
            # Column chunking: uniform CHUNK-wide pieces. (Optionally the
            # last chunk can be split in half to shorten the final
            # ACTIVATE in the kernel tail; measured as a wash, off by
            # default.)  Chunk DMAs alternate between the two HWDGE
            # rings (nc.sync / nc.scalar) so descriptor issue and
            # completion latencies of consecutive transfers overlap --
            # worth ~12us over a single ring at these sizes.
            # Halve the last K_TAIL_HALVE full chunks: their EXPs run
            # after the DMA stream has finished (ACT is the critical
            # path in the kernel tail), and smaller chunks drain that
            # backlog sooner.
            tail_halve = int(_os.environ.get("K_TAIL_HALVE", "0"))
            # Halve the first chunk: the HWDGE descriptor ramp engages
            # engines 6-15 a few us after 0-5, so a short lead-in chunk
            # reaches full 16-engine streaming sooner.
            head_halve = int(_os.environ.get("K_HEAD_HALVE", "1"))
            spans = []  # (tile, col_start, col_len, acc_col)
            acc_cols = 0
            n_full = TILES * NCH
            for t in range(TILES):
                for ci in range(NCH):
                    c0, cl = ci * CHUNK, CHUNK
                    k_full = t * NCH + ci
                    halve = k_full >= n_full - tail_halve or k_full < head_halve
                    if halve and CHUNK % 2 == 0:
                        spans.append((t, c0, cl // 2, acc_cols))
                        acc_cols += 1
                        c0, cl = c0 + cl // 2, cl // 2
                    spans.append((t, c0, cl, acc_cols))
                    acc_cols += 1

            acc = accp.tile([P, acc_cols], mybir.dt.float32)
            dma_engines = [nc.sync, nc.scalar]
            ring_mode = int(_os.environ.get("K_RING_MODE", "4"))
            # With exp_lag=1, each chunk's EXP is emitted after the NEXT
            # chunk's DMA issue — hinting the scheduler to keep the scalar
            # sequencer's DMA issues ahead of its EXP executions.
            exp_lag = int(_os.environ.get("K_EXP_LAG", "0"))
            pending = []  # (tile_handle, col_len, acc_col)
            for k, (t, c0, cl, ak) in enumerate(spans):
                if ring_mode == 1:
                    eng = t  # per row-tile
                elif ring_mode == 2:
                    eng = 0 if k < len(spans) // 2 else 1
                elif ring_mode >= 3:
                    # Alternate rings mid-stream, but issue the last
                    # (ring_mode) chunks from the sync ring: the scalar
                    # sequencer is in-order and busy running the EXPs, so
                    # its late DMA issues otherwise stall the pipe tail.
                    tail_n = ring_mode
                    eng = k if k < len(spans) - tail_n else 0
                else:
                    eng = k  # alternate per chunk
                wt = wfp.tile([P, CHUNK], mybir.dt.float32)
                dma_engines[eng % len(dma_engines)].dma_start(
                    out=wt[:, :cl],
                    in_=wf[t * P : (t + 1) * P, c0 : c0 + cl],
                )
                pending.append((wt, cl, ak))
                while len(pending) > exp_lag:
                    pwt, pcl, pak = pending.pop(0)
                    nc.scalar.activation(
                        out=pwt[:, :pcl],
                        in_=pwt[:, :pcl],
                        func=mybir.ActivationFunctionType.Exp,
                        scale=S,
                        bias=bias_t[:, :],
                        accum_out=acc[:, pak : pak + 1],
                    )
            for pwt, pcl, pak in pending:
                nc.scalar.activation(
                    out=pwt[:, :pcl],
                    in_=pwt[:, :pcl],
                    func=mybir.ActivationFunctionType.Exp,
                    scale=S,
                    bias=bias_t[:, :],
                    accum_out=acc[:, pak : pak + 1],
                )
            for t in range(TILES):
                lo = min(ak for (tt, _, _, ak) in spans if tt == t)
                hi = max(ak for (tt, _, _, ak) in spans if tt == t)
                nc.vector.tensor_reduce(
                    osb[:, t : t + 1],
                    acc[:, lo : hi + 1],
                    axis=mybir.AxisListType.X,
                    op=mybir.AluOpType.add,
                )
            nc.sync.dma_start(out=out[:, :], in_=osb[:, :])
    nc.compile()
    return nc


def _get_graph():
    if "nc" not in _CACHE:
        _CACHE["nc"] = _build_graph()
    return _CACHE["nc"]


def kernel(wf: np.ndarray, labels: np.ndarray) -> np.ndarray:
    global LAST_RESULTS
    from concourse import bass_utils

    wf = np.ascontiguousarray(np.asarray(wf, dtype=np.float32))
    lab = np.asarray(labels).astype(np.int64)
    assert wf.shape == (B, C) and lab.shape == (B,)

    nc = _get_graph()
    in_maps = [{"wf": wf[i * ROWS : (i + 1) * ROWS]} for i in range(NCORES)]
    res = bass_utils.run_bass_kernel_spmd(nc, in_maps, core_ids=list(range(NCORES)))
    LAST_RESULTS = res

    # res.results[i]["out"][p, t] = shifted row sum of shard row t*P + p.
    R = np.concatenate(
        [res.results[i]["out"].T.reshape(ROWS) for i in range(NCORES)]
    ).astype(np.float64)  # [B]

    # Host epilogue: replay the reference's f32 arithmetic exactly.
    rows = np.arange(B)
    tgt = wf[rows, lab]                                     # [B] f32
    with np.errstate(over="ignore", invalid="ignore", divide="ignore"):
        rowsum32 = (np.exp(np.float64(SHIFT)) * R).astype(np.float32)  # inf on overflow
        exp_lab = np.exp(np.float32(S) * tgt)               # f32, inf where S*t>~88.7
        num = np.float32(S) * (tgt - np.float32(M))         # f32
        excl = rowsum32 - exp_lab
        denom = np.exp(num) + excl
        L = num - np.log(denom)
        out = -np.mean(L, dtype=np.float32)
    return np.float32(out)
